# revision 16
# baseline (speedup 1.0000x reference)
# Trainium2 Bass kernel for nn_CNN3_F_P (pairwise conv + 3x conv1d + 2 FC).
# Data parallel over 8 NeuronCores: batch 2048 -> 256 samples/core.
# Self-contained: hardcodes all shapes; host preps DMA-friendly weight layouts.
import sys

import numpy as np

try:
    import concourse.bass as bass  # noqa: F401
except ImportError:
    sys.path.insert(0, "/opt/trn_rl_repo")

import ml_dtypes

import concourse.bass as bass
import concourse.mybir as mybir
import concourse.tile as tile
from concourse import bacc
from concourse.bass_utils import run_bass_kernel_spmd

# Problem shapes
INST, CTX = 64, 128
PC = 256          # pairwise out channels; CH1=CH2=CH3=256
L = CTX - 1       # 127
F1, OUT = 400, 2
B = 2048
N_CORES = 8
BPC = B // N_CORES  # 256 samples per core
GT = 4              # samples per matmul group (free dim GT*L = 508 <= 512)

FP32 = mybir.dt.float32
BF16 = mybir.dt.bfloat16
BF16_NP = ml_dtypes.bfloat16
RELU = mybir.ActivationFunctionType.Relu
ADD = mybir.AluOpType.add
MULT = mybir.AluOpType.mult


def build_nc(n_samples: int, debug: bool = False) -> bass.Bass:
    """Emit the per-core Tile program. Every core runs this same program on
    its own 'n_samples'-sample shard."""
    assert n_samples % (2 * GT) == 0
    n_groups = n_samples // GT
    sb_n = n_samples // 128 if n_samples >= 128 else 1
    sb_sz = min(n_samples, 128)

    nc = bacc.Bacc()

    # DRAM parameters (per-core shard + replicated weights).
    # xt rows 0..63 = x transposed to (inst, sample, pos); rows 64..127 = the
    # pos-0 column broadcast along pos (so the pairwise layer is one matmul).
    xt_d = nc.declare_dram_parameter("xt", [128, n_samples, CTX], BF16, isOutput=False)
    wp_d = nc.declare_dram_parameter("wp", [128, PC], BF16, isOutput=False)
    wcv_d = nc.declare_dram_parameter("wcv", [128, 3, 2, 3, 2, 128], BF16, isOutput=False)
    wf1_d = nc.declare_dram_parameter("wf1", [L, 128, 2, F1], BF16, isOutput=False)
    bf1_d = nc.declare_dram_parameter("bf1", [1, F1], BF16, isOutput=False)
    wf2_d = nc.declare_dram_parameter("wf2", [128, OUT, F1], BF16, isOutput=False)
    bcv_d = nc.declare_dram_parameter("bcv", [128, 8], FP32, isOutput=False)
    bf2_d = nc.declare_dram_parameter("bf2", [128, OUT], FP32, isOutput=False)
    out_d = nc.declare_dram_parameter("out", [n_samples, OUT], FP32, isOutput=True)
    if debug:
        dbg_h = [
            nc.declare_dram_parameter(f"dbg_h{i}", [2, 128, GT, 129], FP32, isOutput=True)
            for i in range(3)
        ]
        dbg_h3 = nc.declare_dram_parameter("dbg_h3", [2, 128, L, n_samples], FP32, isOutput=True)
        dbg_f1 = nc.declare_dram_parameter("dbg_f1", [sb_n, 128, F1], FP32, isOutput=True)

    with tile.TileContext(nc) as tc:
        with (
            tc.tile_pool(name="consts", bufs=1) as consts,
            tc.tile_pool(name="hbuf", bufs=1) as hbuf,
            tc.tile_pool(name="xin", bufs=4) as xin,
        ):
            # ---- resident weights/biases ----
            wp_t = consts.tile([128, PC], BF16, tag="wp", name="wp")
            nc.sync.dma_start(wp_t[:], wp_d[:])
            bcv_t = consts.tile([128, 8], FP32, tag="bcv", name="bcv")
            nc.sync.dma_start(bcv_t[:], bcv_d[:])
            wcv_t = consts.tile([128, 3, 2, 3, 2, 128], BF16, tag="wcv", name="wcv")
            wf2_t = consts.tile([128, OUT, F1], BF16, tag="wf2", name="wf2")
            bf1_t = consts.tile([1, F1], BF16, tag="bf1", name="bf1")
            bf2_t = consts.tile([128, OUT], FP32, tag="bf2", name="bf2")
            ones_t = consts.tile([1, 128], BF16, tag="ones", name="ones")
            nc.vector.memset(ones_t[:], 1.0)

            # ---- persistent activation buffers ----
            # h0..h2: ping-pong per group parity; stripes of 129 cols/sample
            # (col 0 and col 128 are zero pads for the k=3 conv taps).
            hconv = []  # hconv[layer][parity][blk]
            NPAR = [3, 2, 2]
            for layer in range(3):
                byp = []
                for par in range(NPAR[layer]):
                    blks = []
                    for o in range(2):
                        t = hbuf.tile(
                            [128, GT, 129], BF16,
                            tag=f"h{layer}_{par}_{o}", name=f"h{layer}_{par}_{o}",
                        )
                        nc.vector.memset(t[:, :, 0:1], 0.0)
                        nc.vector.memset(t[:, :, 128:129], 0.0)
                        blks.append(t)
                    byp.append(blks)
                hconv.append(byp)
            # h3: conv3 output, transposed [c, l, sample] so fc1's stationary
            # slices are contiguous (FWL-eligible), bf16
            h3 = [
                hbuf.tile([128, L, n_samples], BF16, tag=f"h3_{o}", name=f"h3_{o}")
                for o in range(2)
            ]

            # ---- phase A: pairwise + conv1..conv3 ----
            # Pairwise for group g+2 is emitted ahead of group g's convs so the
            # PE never waits on the h0 relu; relus alternate Scalar (o=0) and
            # Vector (o=1) so both channel blocks finish in parallel.
            MAXALU = mybir.AluOpType.max

            def relu_to(dst, ps, bias_idx, use_dve):
                if use_dve:
                    nc.vector.tensor_scalar(
                        dst, ps, bcv_t[:, bias_idx : bias_idx + 1], 0.0, ADD, MAXALU
                    )
                else:
                    nc.scalar.activation(
                        dst, ps, RELU, bias=bcv_t[:, bias_idx : bias_idx + 1]
                    )

            with (
                tc.tile_pool(name="cpsum", bufs=6, space=bass.MemorySpace.PSUM) as cp,
                tc.tile_pool(name="ppsum", bufs=2, space=bass.MemorySpace.PSUM) as pp,
            ):

                def pairwise(g):
                    par = g % 3
                    s0 = g * GT
                    px = xin.tile([128, GT, CTX], BF16, tag="px", name="px")
                    nc.sync.dma_start(px[:], xt_d[:, s0 : s0 + GT, :])
                    for o in range(2):
                        ps = pp.tile([128, GT, L], FP32, tag="pp", name="pp")
                        nc.tensor.matmul(
                            ps[:],
                            wp_t[:, o * 128 : (o + 1) * 128],
                            px[:, :, 1:CTX],
                            start=True,
                            stop=True,
                        )
                        relu_to(hconv[0][par][o][:, :, 1:128], ps[:], o, o == 1)

                def conv_layer(li, g):
                    rpar = g % 3 if li == 0 else g % 2
                    wpar = g % 2
                    s0 = g * GT
                    for o in range(2):
                        ps = cp.tile([128, GT, L], FP32, tag="cp", name="cp")
                        n_mm = 0
                        for i in range(2):
                            for k in range(3):
                                nc.tensor.matmul(
                                    ps[:],
                                    wcv_t[:, li, i, k, o, :],
                                    hconv[li][rpar][i][:, :, k : k + L],
                                    start=(n_mm == 0),
                                    stop=(n_mm == 5),
                                )
                                n_mm += 1
                        if li < 2:
                            dst = hconv[li + 1][wpar][o][:, :, 1:128]
                            src_ap = ps[:]
                        else:
                            dst = h3[o][:, :, s0 : s0 + GT]
                            src_ap = ps[:].transpose([0, 2, 1])
                        relu_to(dst, src_ap, 2 * (li + 1) + o, o == 1)

                pairwise(0)
                if n_groups > 1:
                    pairwise(1)
                # big consts stream in behind the first input tiles
                for li in range(3):
                    nc.sync.dma_start(wcv_t[:, li], wcv_d[:, li])
                nc.sync.dma_start(wf2_t[:], wf2_d[:])
                nc.sync.dma_start(bf1_t[:], bf1_d[:])
                nc.sync.dma_start(bf2_t[:], bf2_d[:])
                for g in range(n_groups):
                    if g + 2 < n_groups:
                        pairwise(g + 2)
                    for li in range(3):
                        conv_layer(li, g)

            # ---- phase B: fc1 (+relu) and fc2 ----
            # fc1 runs "flipped": stationary = h3 sample-block columns,
            # moving = streamed Wfc1 rows -> psum[sample, f1].
            with (
                tc.tile_pool(name="fpsum", bufs=1, space=bass.MemorySpace.PSUM) as fp,
                tc.tile_pool(name="wstream", bufs=12) as ws,
                tc.tile_pool(name="fout", bufs=1) as fo,
            ):
                f1ps = [
                    fp.tile([sb_sz, F1], FP32, tag=f"f1p{sb}", name=f"f1p{sb}")
                    for sb in range(sb_n)
                ]
                # bias row via a K=1 matmul of ones^T x bfc1
                for sb in range(sb_n):
                    nc.tensor.matmul(
                        f1ps[sb][:],
                        ones_t[:, :sb_sz],
                        bf1_t[:],
                        start=True,
                        stop=False,
                    )
                for l in range(L):
                    wt = ws.tile([128, 2, F1], BF16, tag="wf1", name="wf1_t")
                    nc.sync.dma_start(wt[:], wf1_d[l])
                    for i in range(2):
                        for sb in range(sb_n):
                            nc.tensor.matmul(
                                f1ps[sb][:],
                                h3[i][:, l, sb * 128 : sb * 128 + sb_sz],
                                wt[:, i, :],
                                start=False,
                                stop=(l == L - 1 and i == 1),
                            )
                for sb in range(sb_n):
                    f1o = fo.tile([sb_sz, F1], BF16, tag=f"f1o{sb}", name=f"f1o{sb}")
                    nc.scalar.activation(f1o[:], f1ps[sb][:], RELU)
                    out_t = fo.tile([sb_sz, OUT], FP32, tag=f"out{sb}", name=f"out{sb}")
                    for o in range(OUT):
                        tmp = fo.tile([sb_sz, F1], FP32, tag="tmp", name="tmp", bufs=2)
                        nc.vector.tensor_tensor(tmp[:], f1o[:], wf2_t[:sb_sz, o, :], MULT)
                        nc.vector.tensor_reduce(
                            out_t[:, o : o + 1], tmp[:], mybir.AxisListType.X, ADD
                        )
                    nc.vector.tensor_tensor(out_t[:], out_t[:], bf2_t[:sb_sz, :], ADD)
                    nc.sync.dma_start(out_d[sb * 128 : sb * 128 + sb_sz, :], out_t[:])
                    if debug:
                        df = fo.tile([sb_sz, F1], FP32, tag=f"dbgf{sb}", name=f"dbgf{sb}")
                        nc.vector.tensor_copy(df[:], f1o[:])
                        nc.sync.dma_start(dbg_f1[sb, :sb_sz, :], df[:])
                if debug:
                    for li in range(3):
                        for o in range(2):
                            dh = fo.tile([128, GT, 129], FP32, tag=f"dbg{li}{o}", name=f"dbg{li}{o}")
                            nc.vector.tensor_copy(dh[:], hconv[li][0][o][:])
                            nc.sync.dma_start(dbg_h[li][o], dh[:])
                    for o in range(2):
                        dh3 = fo.tile([128, L, n_samples], FP32, tag=f"dbgh3{o}", name=f"dbgh3{o}")
                        nc.vector.tensor_copy(dh3[:], h3[o][:])
                        nc.sync.dma_start(dbg_h3[o], dh3[:])

    nc.compile()
    return nc


def prep_inputs(x, Wp, bp, W1, b1, W2, b2, W3, b3, Wfc1, bfc1, Wfc2, bfc2):
    """Host-side layout prep (numpy). Returns dict of full-size arrays keyed
    by the kernel's DRAM parameter names; 'xt' still has the full batch."""
    f32 = np.float32
    x, Wp, bp, W1, b1, W2, b2, W3, b3, Wfc1, bfc1, Wfc2, bfc2 = (
        np.asarray(v, dtype=f32)
        for v in (x, Wp, bp, W1, b1, W2, b2, W3, b3, Wfc1, bfc1, Wfc2, bfc2)
    )
    # x: (B, CTX*INST) -> (INST, B, CTX); bottom half = pos-0 col broadcast
    xt_top = np.ascontiguousarray(x.reshape(B, CTX, INST).transpose(2, 0, 1))
    xt_bot = np.broadcast_to(xt_top[:, :, 0:1], (INST, B, CTX))
    xt = np.concatenate([xt_top, xt_bot], axis=0).astype(BF16_NP)  # (128, B, CTX)
    # Wp: (PC, INST, 2) -> (128, PC): rows 0..63 = Wp[:,:,1].T, 64..127 = Wp[:,:,0].T
    wp = np.ascontiguousarray(
        np.concatenate([Wp[:, :, 1].T, Wp[:, :, 0].T], axis=0)
    ).astype(BF16_NP)
    # conv weights: (Cout, Cin, K) -> [cin_in, layer, cin_blk, k, cout_blk, cout_in]
    def conv_t(W):
        A = W.reshape(2, 128, 2, 128, 3)  # [ob, oi, ib, ii, k]
        return A.transpose(3, 2, 4, 0, 1)  # (128, 2, 3, 2, 128)

    wcv = np.ascontiguousarray(
        np.stack([conv_t(W1), conv_t(W2), conv_t(W3)], axis=1)
    ).astype(BF16_NP)
    # Wfc1: (400, 32512) with col = c3*L + l -> (L, cin_in, cin_blk, 400)
    wf1 = np.ascontiguousarray(
        Wfc1.reshape(F1, 2, 128, L).transpose(3, 2, 1, 0)
    ).astype(BF16_NP)
    bf1 = np.ascontiguousarray(bfc1.reshape(1, F1)).astype(BF16_NP)
    # Wfc2 (2, 400) replicated across partitions for the DVE fc2 reduce
    wf2 = np.ascontiguousarray(
        np.broadcast_to(Wfc2[None, :, :], (128, OUT, F1))
    ).astype(BF16_NP)
    bf2 = np.ascontiguousarray(np.broadcast_to(bfc2[None, :], (128, OUT))).astype(f32)
    # conv biases: (128, 8) fp32, col = layer*2 + blk
    bcv = np.ascontiguousarray(
        np.stack([bp, b1, b2, b3]).reshape(4, 2, 128).transpose(2, 0, 1).reshape(128, 8)
    ).astype(f32)
    return {
        "xt": xt,
        "wp": wp,
        "wcv": wcv,
        "wf1": wf1,
        "bf1": bf1,
        "wf2": wf2,
        "bcv": bcv,
        "bf2": bf2,
    }


_NC_CACHE = {}


def _get_nc(n_samples):
    if n_samples not in _NC_CACHE:
        _NC_CACHE[n_samples] = build_nc(n_samples)
    return _NC_CACHE[n_samples]


def run(inputs: dict, trace: bool = False, tmpdir: str | None = None):
    """Run on the 8 NeuronCores. Returns (output (B,2) fp32, exec_time_ns|None)."""
    full = prep_inputs(**inputs)
    xt = full.pop("xt")
    in_maps = []
    for c in range(N_CORES):
        m = dict(full)
        m["xt"] = np.ascontiguousarray(xt[:, c * BPC : (c + 1) * BPC, :])
        in_maps.append(m)
    nc = _get_nc(BPC)
    res = run_bass_kernel_spmd(
        nc,
        in_maps,
        list(range(N_CORES)),
        trace=trace,
        trace_cores=[0] if trace else None,
        tmpdir=tmpdir,
    )
    out = np.concatenate([np.asarray(r["out"]) for r in res.results], axis=0)
    return out.astype(np.float32), res.exec_time_ns


def kernel(**inputs) -> np.ndarray:
    return run(inputs, trace=False)[0]
